# revision 27
# baseline (speedup 1.0000x reference)
"""Trainium2 Bass kernel for nn_AttentionCellEncoder (optimized).

Contract: kernel(**inputs) takes FULL unsharded inputs (as produced by
setup_inputs) and returns the FULL [2048, 256] float32 output. Internally
shards cells across 8 NeuronCores, runs a Bass/Tile kernel via
run_bass_kernel_spmd, and reassembles the output.

Strategy vs the straightforward version:
  * bf16 everywhere on device (tolerance 2e-2; measured end-to-end err ~4e-3).
    Attention matmuls have small free dims where fp32 runs at 1/4 rate.
  * Ragged-aware packing: cells are bin-packed by true length into 128-token
    tiles (up to CMAX cells per tile) instead of one fixed 64-token slot per
    cell; with uniform lengths this drops ~45% of all gather/matmul work.
  * Full-tile scores + multiplicative 0/1 block-diagonal mask, so attention
    uses 128-deep contractions and few large matmuls.
  * Per-input runtime specialization: the kernel is traced/compiled for the
    actual packing of the given cell_len distribution.

Self-contained: all shapes hardcoded; no file I/O.
"""

import numpy as np
import ml_dtypes

import concourse.bass as bass
import concourse.mybir as mybir
import concourse.tile as tile
from concourse import bacc
from concourse.bass_utils import run_bass_kernel_spmd
from concourse.masks import make_identity

FP = mybir.dt.float32
BF = mybir.dt.bfloat16
I32 = mybir.dt.int32
NPBF = ml_dtypes.bfloat16
P = 128

# Problem dims
NUM_HEADS = 8
NUM_CHUNKS, INPUT_DIM = 50000, 768   # D = 768
HIDDEN_DIM, OUTPUT_DIM = 512, 256    # H = 512
NUM_CELLS, MAX_LEN = 2048, 64        # C, L
HEAD_DIM = HIDDEN_DIM // NUM_HEADS   # 64

N_CORES = 8
CMAX = 16                 # max cells packed into one 128-token tile
DCH = INPUT_DIM // P      # 6 d-chunks
HCH = HIDDEN_DIM // P     # 4 h-chunks
TPB = 4                   # tiles per block (512-token QKV blocks)
# wts column offsets
WQ0, WK0, WV0, WF0 = 0, DCH * HIDDEN_DIM, 2 * DCH * HIDDEN_DIM, 3 * DCH * HIDDEN_DIM
WCOLS = 3 * DCH * HIDDEN_DIM + HCH * OUTPUT_DIM  # 9216 + 1024


def build_kernel(T: int, with_q_bias: bool, with_v_bias: bool, repeat: int = 1,
                 stage: int = 99):
    """Trace + compile the per-core SPMD kernel for T tiles/core.

    stage: truncate the per-block body for HW bisection (1=gather+transpose,
    2=+qkv, 3=+scores/exp/mask, 4=+ctx/normalize, 99=full)."""
    assert T % 8 == 0
    nc = bacc.Bacc(None)

    table = nc.dram_tensor("table", [NUM_CHUNKS, INPUT_DIM], BF, kind="ExternalInput")
    wts = nc.dram_tensor("wts", [P, WCOLS], BF, kind="ExternalInput")
    idxs = nc.dram_tensor("idxs", [P, T], I32, kind="ExternalInput")
    bmask = nc.dram_tensor("bmask", [T * P, P], BF, kind="ExternalInput")
    uw = nc.dram_tensor("uw", [T * P, CMAX], BF, kind="ExternalInput")
    if with_q_bias:
        bq_c = nc.dram_tensor("bq_c", [P, HCH], FP, kind="ExternalInput")
    if with_v_bias:
        bv_r = nc.dram_tensor("bv_r", [1, HIDDEN_DIM], BF, kind="ExternalInput")
    out = nc.dram_tensor("out", [T * CMAX, OUTPUT_DIM], FP, kind="ExternalOutput")

    with tile.TileContext(nc) as tc:
        with (
            tc.tile_pool(name="const", bufs=1) as cpool,
            tc.tile_pool(name="xp", bufs=3) as xpool,
            tc.tile_pool(name="blk", bufs=2) as bpool,
            tc.tile_pool(name="sm", bufs=2) as spool,
            tc.tile_pool(name="op", bufs=2) as opool,
            tc.tile_pool(name="ps", bufs=2, space="PSUM") as pspool,
        ):
            ident = cpool.tile([P, P], BF)
            make_identity(nc, ident[:])
            wsb = cpool.tile([P, WCOLS], BF)
            nc.sync.dma_start(out=wsb[:], in_=wts[:, :])
            idx_sb = cpool.tile([P, T], I32)
            nc.sync.dma_start(out=idx_sb[:], in_=idxs[:, :])
            poolsb = cpool.tile([P, T * HCH * CMAX], BF)
            if stage < 99:
                nc.gpsimd.memset(poolsb[:], 0.0)
            if with_q_bias:
                bq_sb = cpool.tile([P, HCH], FP)
                nc.sync.dma_start(out=bq_sb[:], in_=bq_c[:, :])
            if with_v_bias:
                ones1 = cpool.tile([1, P], BF)
                nc.gpsimd.memset(ones1[:], 1.0)
                bv_sb = cpool.tile([1, HIDDEN_DIM], BF)
                nc.sync.dma_start(out=bv_sb[:], in_=bv_r[:, :])

            def gather_block(b):
                """Issue the 4 indirect row-gathers of block b (prefetch)."""
                xs = []
                for t4 in range(TPB):
                    t = b * TPB + t4
                    x = xpool.tile([P, INPUT_DIM], BF, tag="x", bufs=9)
                    nc.gpsimd.indirect_dma_start(
                        out=x[:], out_offset=None, in_=table[:],
                        in_offset=bass.IndirectOffsetOnAxis(
                            ap=idx_sb[:, t:t + 1], axis=0),
                    )
                    xs.append(x)
                return xs

            def transpose_block(b, xs):
                """Transpose block b's gathered tiles to d-major xT."""
                xT = bpool.tile([P, DCH * TPB * P], BF, tag="xT")
                for t4 in range(TPB):
                    x = xs[t4]
                    pa = pspool.tile([P, DCH * P], BF, tag="xp")
                    for j in range(DCH):
                        nc.tensor.transpose(
                            out=pa[:, j * P:(j + 1) * P],
                            in_=x[:, j * P:(j + 1) * P],
                            identity=ident[:])
                    nc.vector.tensor_copy(
                        out=xT[:].rearrange("p (j n) -> p j n", j=DCH)
                            [:, :, t4 * P:(t4 + 1) * P],
                        in_=pa[:].rearrange("p (j n) -> p j n", j=DCH),
                    )
                return xT

            def qk_proj(xT):
                """qT/kT [128 = 2 heads x 64 hd, hc*512 + tok] + odd-head
                copies at partition base 0 (mixed-quadrant matmuls abort on
                HW, so scores always use base-0 operands)."""
                qT = bpool.tile([P, HCH * TPB * P], BF, tag="qT")
                kT = bpool.tile([P, HCH * TPB * P], BF, tag="kT")
                qTo = bpool.tile([64, HCH * TPB * P], BF, tag="qTo")
                kTo = bpool.tile([64, HCH * TPB * P], BF, tag="kTo")
                for (w0, dst, on_act) in ((WQ0, qT, True), (WK0, kT, False)):
                    for hc in range(HCH):
                        acc = pspool.tile([P, TPB * P], FP, tag="acc")
                        for j in range(DCH):
                            nc.tensor.matmul(
                                out=acc[:],
                                lhsT=wsb[:, w0 + j * HIDDEN_DIM + hc * P:
                                         w0 + j * HIDDEN_DIM + (hc + 1) * P],
                                rhs=xT[:, j * TPB * P:(j + 1) * TPB * P],
                                start=(j == 0), stop=(j == DCH - 1),
                            )
                        d = dst[:, hc * TPB * P:(hc + 1) * TPB * P]
                        if on_act:
                            if with_q_bias:
                                nc.scalar.activation(
                                    out=d, in_=acc[:],
                                    func=mybir.ActivationFunctionType.Identity,
                                    bias=bq_sb[:, hc:hc + 1])
                            else:
                                nc.scalar.activation(
                                    out=d, in_=acc[:],
                                    func=mybir.ActivationFunctionType.Copy)
                        else:
                            nc.vector.tensor_copy(out=d, in_=acc[:])
                nc.sync.dma_start(out=qTo[0:64, :], in_=qT[64:P, :])
                nc.sync.dma_start(out=kTo[0:64, :], in_=kT[64:P, :])
                return qT, kT, qTo, kTo

            VW = HEAD_DIM + 1   # per-head v block: 64 ctx cols + 1 ones col

            def v_tile(xT, v, t4):
                """v[:, t4*520 + h*65 + (0:64)] = x_tile @ Wv (+bias); col 64
                of each head block is 1.0 so ctx and the softmax denominator
                come out of a single matmul per head."""
                acc = pspool.tile([P, HIDDEN_DIM], FP, tag="acc")
                nmm = DCH + (1 if with_v_bias else 0)
                for j in range(DCH):
                    nc.tensor.matmul(
                        out=acc[:],
                        lhsT=xT[:, j * TPB * P + t4 * P:
                                j * TPB * P + (t4 + 1) * P],
                        rhs=wsb[:, WV0 + j * HIDDEN_DIM:
                                WV0 + (j + 1) * HIDDEN_DIM],
                        start=(j == 0), stop=(j == nmm - 1),
                    )
                if with_v_bias:
                    nc.tensor.matmul(out=acc[:], lhsT=ones1[0:1, :],
                                     rhs=bv_sb[0:1, :], start=False, stop=True)
                vv = v[:, t4 * NUM_HEADS * VW:(t4 + 1) * NUM_HEADS * VW]
                vv = vv.rearrange("p (h e) -> p h e", h=NUM_HEADS)
                nc.scalar.activation(
                    out=vv[:, :, 0:HEAD_DIM],
                    in_=acc[:].rearrange("p (h d) -> p h d", h=NUM_HEADS),
                    func=mybir.ActivationFunctionType.Copy)
                nc.gpsimd.memset(vv[:, :, HEAD_DIM:VW], 1.0)

            def att1(st):
                """scores -> exp -> 0/1-mask for block st['b']; fills
                st['ems'], st['us']."""
                b, qT, kT, qTo, kTo = st["b"], st["qT"], st["kT"], st["qTo"], st["kTo"]
                for t4 in range(TPB):
                    t = b * TPB + t4
                    B = spool.tile([P, P], BF, tag="B")
                    nc.sync.dma_start(out=B[:], in_=bmask[t * P:(t + 1) * P, :])
                    u_sb = spool.tile([P, CMAX], BF, tag="u", bufs=5)
                    nc.sync.dma_start(out=u_sb[:], in_=uw[t * P:(t + 1) * P, :])
                    st["us"].append(u_sb)
                    e = spool.tile([P, NUM_HEADS * P], BF, tag="e")
                    for half in range(2):
                        sc = pspool.tile([P, 4 * P], FP, tag="sc")
                        for hh in range(4):
                            h = half * 4 + hh
                            kk, qq = (kT, qT) if h % 2 == 0 else (kTo, qTo)
                            col = (h // 2) * TPB * P + t4 * P
                            nc.tensor.matmul(
                                out=sc[:, hh * P:(hh + 1) * P],
                                lhsT=kk[0:64, col:col + P],
                                rhs=qq[0:64, col:col + P],
                                start=True, stop=True,
                            )
                        nc.scalar.activation(
                            out=e[:, half * 4 * P:(half + 1) * 4 * P],
                            in_=sc[:],
                            func=mybir.ActivationFunctionType.Exp)
                    em = spool.tile([P, NUM_HEADS * P], BF, tag="em", bufs=5)
                    nc.vector.tensor_tensor(
                        out=em[:].rearrange("p (h l) -> p h l", h=NUM_HEADS),
                        in0=e[:].rearrange("p (h l) -> p h l", h=NUM_HEADS),
                        in1=B[:, None, :].to_broadcast([P, NUM_HEADS, P]),
                        op=mybir.AluOpType.mult,
                    )
                    st["ems"].append(em)

            def att2_tile(st, t4):
                """ctx/den -> normalize -> pool for tile t4 of block st['b']."""
                b, v, em, u_sb = st["b"], st["v"], st["ems"][t4], st["us"][t4]
                t = b * TPB + t4
                # heads 0-6 fused [ctx|den] at h*65 (all inside PSUM bank 0);
                # head 7 at col 512 (bank 1 start) — a matmul output must not
                # cross a 2KB PSUM bank boundary.
                H7 = 512
                cd = pspool.tile([P, H7 + VW], FP, tag="cd", bufs=1)
                for h in range(NUM_HEADS):
                    o0 = h * VW if h < 7 else H7
                    nc.tensor.matmul(
                        out=cd[:, o0:o0 + VW],
                        lhsT=em[:, h * P:(h + 1) * P],
                        rhs=v[:, t4 * NUM_HEADS * VW + h * VW:
                              t4 * NUM_HEADS * VW + (h + 1) * VW],
                        start=True, stop=True,
                    )
                cdv = cd[:, 0:7 * VW].rearrange("p (h e) -> p h e", h=7)
                r = spool.tile([P, NUM_HEADS], FP, tag="r")
                nc.vector.reciprocal(out=r[:, 0:7, None],
                                     in_=cdv[:, :, HEAD_DIM:VW])
                nc.vector.reciprocal(out=r[:, 7:8],
                                     in_=cd[:, H7 + HEAD_DIM:H7 + VW])
                cn = spool.tile([P, HIDDEN_DIM], BF, tag="cn")
                nc.vector.tensor_tensor(
                    out=cn[:, 0:7 * HEAD_DIM]
                        .rearrange("p (h d) -> p h d", h=7),
                    in0=cdv[:, :, 0:HEAD_DIM],
                    in1=r[:, 0:7, None].to_broadcast([P, 7, HEAD_DIM]),
                    op=mybir.AluOpType.mult,
                )
                nc.vector.tensor_tensor(
                    out=cn[:, 7 * HEAD_DIM:HIDDEN_DIM],
                    in0=cd[:, H7:H7 + HEAD_DIM],
                    in1=r[:, 7:8].to_broadcast([P, HEAD_DIM]),
                    op=mybir.AluOpType.mult,
                )
                pt = pspool.tile([P, HCH * CMAX], FP, tag="xp")
                for hc in range(HCH):
                    nc.tensor.matmul(
                        out=pt[:, hc * CMAX:(hc + 1) * CMAX],
                        lhsT=cn[:, hc * P:(hc + 1) * P],
                        rhs=u_sb[:],
                        start=True, stop=True,
                    )
                # poolsb layout: [p, g, hc, slot] with slot = tl*16+j
                g, tl = t // 8, t % 8
                dst = poolsb[:, g * 8 * HCH * CMAX:(g + 1) * 8 * HCH * CMAX]
                dst = dst.rearrange("p (h s) -> p h s", h=HCH)
                nc.vector.tensor_copy(
                    out=dst[:, :, tl * CMAX:(tl + 1) * CMAX],
                    in_=pt[:].rearrange("p (h j) -> p h j", h=HCH))

            NB = T // TPB
            for _rep in range(repeat):
                # one-block software pipeline: attention of block i-1 overlaps
                # the gather/transpose/projections of block i, so the PE never
                # waits on the exp/mask round-trip through ACT/DVE. Gathers
                # are prefetched one block ahead of the transposes.
                prev = None
                xs = gather_block(0)
                for i in range(NB + 1):
                    if i < NB:
                        if i + 1 < NB:
                            xs_next = gather_block(i + 1)
                        xT = transpose_block(i, xs)
                        xs = xs_next
                    if prev is not None and stage >= 3:
                        att1(prev)
                    if i < NB and stage >= 2:
                        qT, kT, qTo, kTo = qk_proj(xT)
                        v = bpool.tile([P, TPB * NUM_HEADS * VW], BF, tag="v")
                    for t4 in range(TPB):
                        if prev is not None and stage >= 4:
                            att2_tile(prev, t4)
                        if i < NB and stage >= 2:
                            v_tile(xT, v, t4)
                    if i < NB and stage >= 2:
                        prev = {"b": i, "qT": qT, "kT": kT, "qTo": qTo,
                                "kTo": kTo, "v": v, "ems": [], "us": []}

                # ---- final projection per group of 8 tiles (128 cell slots) ----
                for g in range(T // 8):
                    acc = pspool.tile([P, OUTPUT_DIM], FP, tag="acc")
                    pg0 = g * 8 * HCH * CMAX
                    for hc in range(HCH):
                        nc.tensor.matmul(
                            out=acc[:], lhsT=poolsb[:, pg0 + hc * P:pg0 + (hc + 1) * P],
                            rhs=wsb[:, WF0 + hc * OUTPUT_DIM:
                                    WF0 + (hc + 1) * OUTPUT_DIM],
                            start=(hc == 0), stop=(hc == HCH - 1),
                        )
                    osb = opool.tile([P, OUTPUT_DIM], FP, tag="osb")
                    nc.scalar.activation(out=osb[:], in_=acc[:],
                                         func=mybir.ActivationFunctionType.Copy)
                    nc.sync.dma_start(out=out[g * P:(g + 1) * P, :], in_=osb[:])

    nc.compile()
    return nc


def pack_cells(lens: np.ndarray):
    """Assign cells to cores and bin-pack each core's cells into 128-token
    tiles (<= CMAX cells/tile). Returns (packs, T): packs[core] = list of
    bins, each bin a list of cell ids; T = uniform tile count per core."""
    order = np.argsort(-lens, kind="stable")
    core_tokens = np.zeros(N_CORES, np.int64)
    core_cells: list[list[int]] = [[] for _ in range(N_CORES)]
    for c in order:
        k = int(np.argmin(core_tokens))
        core_cells[k].append(int(c))
        core_tokens[k] += lens[c]
    packs = []
    for k in range(N_CORES):
        bins: list[list] = []   # [remaining, count, cells]
        for c in core_cells[k]:  # desc length order
            L = int(lens[c])
            for bn in bins:
                if bn[0] >= L and bn[1] < CMAX:
                    bn[0] -= L
                    bn[1] += 1
                    bn[2].append(c)
                    break
            else:
                bins.append([P - L, 1, [c]])
        packs.append([bn[2] for bn in bins])
    T = max(len(p) for p in packs)
    T = ((T + 7) // 8) * 8
    return packs, T


def preprocess(chunk_features, Wq, bq, Wk, bk, Wv, bv, W_in, b_in, Wo, bo,
               Wout, bout, cell_idx, cell_len):
    """Host-side weight folding, cell packing, per-core input maps.

    Returns (in_maps, b_final, slot_of_cell [2048] -> (core, row), T,
    with_q_bias, with_v_bias)."""
    f32 = np.float32
    cf = np.asarray(chunk_features, f32)
    Wq, Wk, Wv = (np.asarray(w, f32) for w in (Wq, Wk, Wv))
    bq, bk, bv = (np.asarray(x, f32) for x in (bq, bk, bv))
    W_in = np.asarray(W_in, f32)
    b_in = np.asarray(b_in, f32)
    Wo, bo = np.asarray(Wo, f32), np.asarray(bo, f32)
    Wout, bout = np.asarray(Wout, f32), np.asarray(bout, f32)

    Wiq, Wik, Wiv = np.split(W_in, 3, axis=0)
    biq, bik, biv = np.split(b_in, 3)
    scale = f32(1.0 / np.sqrt(HEAD_DIM))
    wq_eff = (Wiq @ Wq) * scale          # [512, 768]
    wk_eff = Wik @ Wk
    wv_eff = Wiv @ Wv
    bq_eff = (Wiq @ bq + biq) * scale    # [512]; k-bias is softmax-invariant
    bv_eff = Wiv @ bv + biv
    wfin = Wout @ Wo                     # [256, 512]
    b_final = bo @ Wout.T + bout         # [256]
    with_q_bias = bool(np.any(bq_eff != 0))
    with_v_bias = bool(np.any(bv_eff != 0))

    # wts packing: [128, WCOLS] bf16; w*_sb[p, j*512 + h] = w_eff.T[j*128+p, h]
    wts = np.zeros((P, WCOLS), NPBF)
    for w0, w_eff in ((WQ0, wq_eff), (WK0, wk_eff), (WV0, wv_eff)):
        wt = np.ascontiguousarray(w_eff.T)          # [768, 512]
        for j in range(DCH):
            wts[:, w0 + j * HIDDEN_DIM:w0 + (j + 1) * HIDDEN_DIM] = \
                wt[j * P:(j + 1) * P, :].astype(NPBF)
    wft = np.ascontiguousarray(wfin.T)              # [512, 256]
    for hc in range(HCH):
        wts[:, WF0 + hc * OUTPUT_DIM:WF0 + (hc + 1) * OUTPUT_DIM] = \
            wft[hc * P:(hc + 1) * P, :].astype(NPBF)

    table_b = cf.astype(NPBF)
    ci = np.asarray(cell_idx).astype(np.int32)             # [2048, 64]
    ln = np.maximum(np.asarray(cell_len).astype(np.int64), 1)
    ln = np.minimum(ln, MAX_LEN).astype(np.int32)          # [2048]

    packs, T = pack_cells(ln)

    slot_core = np.zeros(NUM_CELLS, np.int32)
    slot_row = np.zeros(NUM_CELLS, np.int32)
    in_maps = []
    for core in range(N_CORES):
        bins = packs[core]
        idxs = np.zeros((P, T), np.int32)
        bm = np.zeros((T, P, P), NPBF)
        u = np.zeros((T, P, CMAX), NPBF)
        for t in range(T):
            pos = 0
            if t < len(bins):
                for j, c in enumerate(bins[t]):
                    L = int(ln[c])
                    idxs[pos:pos + L, t] = ci[c, :L]
                    bm[t, pos:pos + L, pos:pos + L] = NPBF(1.0)
                    u[t, pos:pos + L, j] = NPBF(1.0 / L)
                    slot_core[c] = core
                    slot_row[c] = t * CMAX + j
                    pos += L
            # padding slots: self-attend so the softmax denominator stays > 0
            for l in range(pos, P):
                bm[t, l, l] = NPBF(1.0)
        m = {
            "table": table_b, "wts": wts, "idxs": idxs,
            "bmask": bm.reshape(T * P, P), "uw": u.reshape(T * P, CMAX),
        }
        if with_q_bias:
            m["bq_c"] = np.ascontiguousarray(bq_eff.reshape(HCH, P).T)
        if with_v_bias:
            m["bv_r"] = bv_eff.reshape(1, HIDDEN_DIM).astype(NPBF)
        in_maps.append(m)
    return in_maps, b_final, (slot_core, slot_row), T, with_q_bias, with_v_bias


_NC_CACHE: dict = {}


def get_nc(T: int, with_q_bias: bool, with_v_bias: bool):
    key = (T, with_q_bias, with_v_bias)
    if key not in _NC_CACHE:
        _NC_CACHE[key] = build_kernel(T, with_q_bias, with_v_bias)
    return _NC_CACHE[key]


def kernel(**inputs) -> np.ndarray:
    in_maps, b_final, (slot_core, slot_row), T, wqb, wvb = preprocess(**inputs)
    nc = get_nc(T, wqb, wvb)
    res = run_bass_kernel_spmd(nc, in_maps, list(range(N_CORES)))
    outs = [np.asarray(res.results[i]["out"]) for i in range(N_CORES)]
    full = np.empty((NUM_CELLS, OUTPUT_DIM), np.float32)
    for c in range(NUM_CELLS):
        full[c] = outs[slot_core[c]][slot_row[c]]
    return (full + b_final[None, :]).astype(np.float32)
